# revision 1
# baseline (speedup 1.0000x reference)
"""Multi-head self-attention (B=4, S=2048, D=1024, H=16) on 8 NeuronCores.

Sharding: tensor-parallel over heads. Core c owns heads {2c, 2c+1} = 128
columns of Wq/Wk/Wv and 128 rows of Wo. Each core computes Q^T/K^T/V for its
two heads over all tokens, runs attention for its 8 (batch, head) pairs, and
produces a partial output O_c = A_c @ Wo_c.  The all-reduce over the 8
partials is done on the host during unsharding.

On-chip layout (per batch b of 2048 tokens):
  - QKV^T tiles [128, 2048] (head-dim on partitions) from X^T resident chunks
  - V is PE-transposed back to token-major and augmented with a ones column,
    so the attention matmul accumulates both U^T = V^T P and the softmax
    denominators in one PSUM tile (row 64).
  - scores are computed transposed (S^T = K Q^T) with the two heads packed
    into disjoint PE row groups; one fused exp over both heads' PSUM banks.
  - no max-subtraction: scores ~ N(0,1) after the 1/sqrt(d) scale, |s| < ~7.

The softmax normalization is a 3-stage software pipeline over the global
(batch, q-tile) iteration index so the vector engine's strict-FIFO queue
never head-blocks on the DMA broadcast roundtrip (which stalled the PE for
~5.5us/iteration and re-throttled HAM):
  t:   U stop -> usb copy (PSUM->SBUF), d-rows -> DRAM, gather [128,8]
  t+1: reciprocal on [128,8] (batched: 32 slow [1,512] recips -> 16 tiny
       ones), scatter to DRAM, broadcast [64,512] per head
  t+2: aT = usb * bc muls, out-projection matmuls, output DMA (bf16)
"""
import os
import sys

for _p in ("/opt/trn_rl_repo", "/root/.axon_site/_ro/trn_rl_repo"):
    if os.path.isdir(_p) and _p not in sys.path:
        sys.path.append(_p)

from contextlib import ExitStack

import numpy as np
import ml_dtypes

import concourse.bass as bass
import concourse.tile as tile
from concourse import mybir
from concourse.bass_utils import run_bass_kernel_spmd
from concourse.masks import make_identity

BF16 = mybir.dt.bfloat16
F32 = mybir.dt.float32
EXP = mybir.ActivationFunctionType.Exp
NP_BF16 = ml_dtypes.bfloat16

B, S, D = 4, 2048, 1024
H, HD = 16, 64
N_CORES = 8
T = B * S  # 8192 tokens
KC = D // 128  # 8 contraction chunks
SCALE = 1.0 / np.sqrt(HD)

# ---------------------------------------------------------------------------
# Tile patches: this walrus build rejects instructions with more than one
# sync wait ("Too many sync wait commands"), so split extra waits into
# preceding same-engine nops, and replace the kernel-tail drain's wait list
# with a chain of single-wait SP nops.
# ---------------------------------------------------------------------------
_MAX_WAITS = 1
_patched = False


def _install_tile_patches():
    global _patched
    if _patched:
        return
    _patched = True
    from concourse.vector_clock import ScopedClock, VectorClock

    orig_lower = tile.TileContext._lower_ordered_insts

    def split_inst_waits(self, ordered):
        for bb_name in list(ordered.keys()):
            insts = ordered[bb_name]
            new = []
            for inst in insts:
                si = inst.sync_info
                if si is not None and len(si.on_wait) > _MAX_WAITS:
                    waits = list(si.on_wait)
                    head, tail = waits[:-_MAX_WAITS], waits[-_MAX_WAITS:]
                    for w in head:
                        nop = mybir.InstNoOp(
                            name=f"ws-{self.nc.next_id()}",
                            engine=inst.engine,
                            bass_nofuse=True,
                        )
                        nop.sync_info = mybir.SyncInfo(on_wait=[w], on_update=[])
                        new.append(nop)
                    inst.sync_info = mybir.SyncInfo(
                        on_wait=tail, on_update=list(si.on_update)
                    )
                new.append(inst)
            ordered[bb_name] = new
        return orig_lower(self, ordered)

    def split_drain_and_barrier(self, tick_clock, wait_clock):
        gc = tick_clock.global_clock
        ticks = eval(repr(gc).replace("VectorClock", ""))
        procs = [(i, t) for i, t in enumerate(ticks) if t > 0]
        for i in range(0, len(procs), _MAX_WAITS):
            chunk = procs[i : i + _MAX_WAITS]
            nop = self.nc.sync.nop(nofuse=True, hint="drain_wait_split")
            pc = VectorClock()
            for proc, tick in chunk:
                pc.require_at_least(proc, tick)
            wait_clock.add_sem_waits(nop.ins, ScopedClock({None: pc}))
        drain_inst = self.nc.sync.drain()
        wait_clock.add_sem_waits(
            drain_inst.ins, ScopedClock({None: gc}), ScopedClock({None: gc.copy()})
        )
        self.nc.all_engine_barrier()
        assert self.sems is not None
        popped = self.nc._tile_sem_poison_stack.pop()
        assert popped is self._sem_poison
        self.nc.clear_and_free_semaphores(list(self.sems.allocated().values()))
        self.nc.all_engine_barrier()

    tile.TileContext._lower_ordered_insts = split_inst_waits
    tile.TileContext._drain_and_barrier = split_drain_and_barrier


# ---------------------------------------------------------------------------
# Device kernel
# ---------------------------------------------------------------------------
def build_attention_nc(with_bias=True, probe=None, out_bf16=True):
    _install_tile_patches()
    nc = bass.Bass()

    xT = nc.declare_dram_parameter("xT", [KC, 128, T], BF16, isOutput=False)
    # weights partition-major [128, KC, 128] so the load DMA is contiguous
    # per partition (host pre-transposes)
    wq = nc.declare_dram_parameter("wq", [128, KC, 128], BF16, isOutput=False)
    wk = nc.declare_dram_parameter("wk", [128, KC, 128], BF16, isOutput=False)
    wv = nc.declare_dram_parameter("wv", [128, KC, 128], BF16, isOutput=False)
    if with_bias:
        bq = nc.declare_dram_parameter("bq", [128], BF16, isOutput=False)
        bk = nc.declare_dram_parameter("bk", [128], BF16, isOutput=False)
        bv = nc.declare_dram_parameter("bv", [128], BF16, isOutput=False)
    else:
        bq = bk = bv = None
    wo = nc.declare_dram_parameter("wo", [128, D], BF16, isOutput=False)
    out = nc.declare_dram_parameter(
        "out", [T, D], BF16 if out_bf16 else F32, isOutput=True
    )

    with tile.TileContext(nc) as tc, ExitStack() as ctx:
        singles = ctx.enter_context(tc.tile_pool(name="singles", bufs=1))
        px = ctx.enter_context(tc.tile_pool(name="px", bufs=16))
        pqk = ctx.enter_context(tc.tile_pool(name="pqk", bufs=2))
        pv = ctx.enter_context(tc.tile_pool(name="pv", bufs=2))
        pa = ctx.enter_context(tc.tile_pool(name="pa", bufs=3))
        ppt = ctx.enter_context(tc.tile_pool(name="ppt", bufs=4))
        pusb = ctx.enter_context(tc.tile_pool(name="pusb", bufs=8))
        pbc = ctx.enter_context(tc.tile_pool(name="pbc", bufs=6))
        pdg = ctx.enter_context(tc.tile_pool(name="pdg", bufs=4))
        pob = ctx.enter_context(tc.tile_pool(name="pob", bufs=3))
        dsc = ctx.enter_context(tc.tile_pool(name="dsc", bufs=8, space="DRAM"))
        psA = ctx.enter_context(tc.tile_pool(name="psA", bufs=2, space="PSUM"))
        psS = ctx.enter_context(tc.tile_pool(name="psS", bufs=2, space="PSUM"))
        psU = ctx.enter_context(tc.tile_pool(name="psU", bufs=2, space="PSUM"))

        # per-batch state created by the A-slices
        state = [dict() for _ in range(B)]
        # per-(b,qt) normalization pipeline state, keyed by global iter idx
        norm = {}

        def a_allocx(b, half=None):
            """Kick off batch b's x DMAs.  The DMA queue shares bandwidth
            fairly across everything enqueued, so emission time IS the
            prioritization: callers stage these so chunks land just before
            the beats that read them.  half=0/1 emits one token-half."""
            st = state[b]
            if "x" not in st:
                st["x"] = [
                    px.tile([128, S], BF16, tag="x", name=f"x_{b}_{kc}")
                    for kc in range(KC)
                ]
            if half is None:
                for kc in range(KC):
                    nc.sync.dma_start(st["x"][kc], xT[kc, :, b * S : (b + 1) * S])
            else:
                c0 = half * (S // 2)
                for kc in range(KC):
                    nc.sync.dma_start(
                        st["x"][kc][:, c0 : c0 + S // 2],
                        xT[kc, :, b * S + c0 : b * S + c0 + S // 2],
                    )

        def a_alloct(b):
            """Allocate batch b's projection tiles (ring reuse waits on the
            previous-but-one batch's last reads — call this only once those
            are in flight)."""
            st = state[b]
            for name in ("q", "k", "v"):
                pool = pqk if name != "v" else pv
                st[name] = pool.tile([128, S], BF16, tag=f"{name}T", name=f"{name}T_{b}")
            st["vS0"] = pv.tile([128, 16, 65], BF16, tag="vS0", name=f"vS0_{b}")
            st["vS1"] = pv.tile([128, 16, 65], BF16, tag="vS1", name=f"vS1_{b}")
            nc.vector.memset(st["vS0"][:, :, 64:65], 1.0)
            nc.vector.memset(st["vS1"][:, :, 64:65], 1.0)

        # --- DMA order tuned for time-to-first-matmul: wq first (small,
        # needed first), then batch-0 x chunks, then the rest ---------------
        w_sb = {}
        b_sb = {}

        def load_w(name, wd, bd):
            w_t = singles.tile([128, KC, 128], BF16, tag=f"w{name}", name=f"w_{name}")
            nc.sync.dma_start(w_t, wd[:, :, :])
            w_sb[name] = w_t
            if with_bias:
                b_t = singles.tile([1, 128], BF16, tag=f"b{name}", name=f"b_{name}")
                nc.sync.dma_start(b_t, bd[:][None, :])
                b_sb[name] = b_t

        load_w("q", wq, bq)
        a_allocx(0, half=0)
        a_alloct(0)
        load_w("k", wk, bk)
        load_w("v", wv, bv)
        wo_sb = singles.tile([128, D], BF16, tag="wo")
        nc.sync.dma_start(wo_sb, wo[:, :])
        if with_bias:
            ones_sb = singles.tile([1, 512], BF16, tag="ones")
            nc.vector.memset(ones_sb, 1.0)
        ident = singles.tile([128, 128], BF16, tag="ident")
        make_identity(nc, ident)

        def a_pieces(b, qt, sel="kvq"):
            """Filler pieces for 1/4 of batch b's QKV projections, one PE
            matmul per piece.  sel picks which projections (k implies the
            V transposes ride with v)."""
            st = state[b]
            pieces = []

            def proj_mm(name, kc, ps_ref):
                def run():
                    if kc == 0:
                        ps_ref.append(psA.tile([128, 512], F32, tag="psA", name=f"ps_{b}_{qt}_{name}"))
                    nc.tensor.matmul(
                        ps_ref[0],
                        w_sb[name][:, kc, :],
                        st["x"][kc][:, qt * 512 : (qt + 1) * 512],
                        start=(kc == 0),
                        stop=(not with_bias and kc == KC - 1),
                    )
                    if kc == KC - 1:
                        if with_bias:
                            nc.tensor.matmul(
                                ps_ref[0], b_sb[name], ones_sb,
                                start=False, stop=True,
                            )
                        nc.vector.tensor_copy(
                            st[name][:, qt * 512 : (qt + 1) * 512], ps_ref[0]
                        )
                return run

            def tr(t):
                def run():
                    tp = psA.tile([128, 128], BF16, tag="psA", name=f"tp_{b}_{t}")
                    nc.tensor.transpose(
                        tp, st["v"][:, t * 128 : (t + 1) * 128], ident
                    )
                    nc.vector.tensor_copy(st["vS0"][:, t, 0:64], tp[:, 0:64])
                    nc.vector.tensor_copy(st["vS1"][:, t, 0:64], tp[:, 64:128])
                return run

            # order matters: each transpose's psA ring-slot predecessor is
            # freed well before it issues (tr0/tr1 reuse the k/v chain
            # slots just after their casts; tr2/tr3 reuse tr0/tr1 a dozen
            # pieces later) so no transpose head-blocks the PE FIFO on a
            # pending DVE copy.
            if "k" in sel:
                ps_ref = []
                for kc in range(KC):
                    pieces.append(proj_mm("k", kc, ps_ref))
            if "v" in sel:
                ps_ref = []
                for kc in range(KC):
                    pieces.append(proj_mm("v", kc, ps_ref))
                pieces.append(tr(qt * 4))
                pieces.append(tr(qt * 4 + 1))
            if "q" in sel:
                ps_ref = []
                for kc in range(KC):
                    pieces.append(proj_mm("q", kc, ps_ref))
            if "v" in sel:
                pieces.append(tr(qt * 4 + 2))
                pieces.append(tr(qt * 4 + 3))
            return pieces

        def norm_back(it):
            """Norm stage-1: batched reciprocal + scatter + broadcasts."""
            def run():
                nst = norm[it]
                rg = pdg.tile([128, 8], F32, tag="rg")
                nc.vector.reciprocal(rg, nst["dg"])
                di2 = dsc.tile([2, 512], F32, tag="di2")
                nc.sync.dma_start(
                    di2[:, :].rearrange("a (x c) -> (a x) c", x=64), rg
                )
                for h in range(2):
                    bc = pbc.tile([64, 512], F32, tag="bc")
                    nc.sync.dma_start(
                        bc, di2[h : h + 1, :].to_broadcast((64, 512))
                    )
                    nst[f"bc{h}"] = bc
            return [run]

        def c_pieces(b, j, it):
            """Norm stage-2 (muls) + output projection for (b, q-tile j)."""
            aref = []

            def muls():
                nst2 = norm.pop(it)
                aref.append(pa.tile([128, 512], BF16, tag="aT", name=f"aT_{b}_{j}"))
                for h in range(2):
                    nc.vector.tensor_mul(
                        aref[0][h * 64 : (h + 1) * 64, :],
                        nst2[f"usb{h}"][0:64, :],
                        nst2[f"bc{h}"],
                    )

            pieces = [muls]

            def op(tt, g, ob_ref):
                def run():
                    if g == 0:
                        ob_ref.append(
                            pob.tile(
                                [128, 1024],
                                BF16 if out_bf16 else F32,
                                tag="ob",
                                name=f"ob_{b}_{tt}",
                            )
                        )
                    col = (tt - 4 * j) * 128
                    po = psA.tile([128, 512], F32, tag="psA")
                    nc.tensor.matmul(
                        po,
                        aref[0][:, col : col + 128],
                        wo_sb[:, g * 512 : (g + 1) * 512],
                        start=True,
                        stop=True,
                    )
                    nc.vector.tensor_copy(
                        ob_ref[0][:, g * 512 : (g + 1) * 512], po
                    )
                    if g == 1:
                        nc.sync.dma_start(
                            out[b * S + tt * 128 : b * S + (tt + 1) * 128, :],
                            ob_ref[0],
                        )
                return run

            for tt in range(4 * j, 4 * j + 4):
                ob_ref = []
                for g in range(2):
                    pieces.append(op(tt, g, ob_ref))
            return pieces

        def av(st, u0, u1, pt, kc):
            nc.tensor.matmul(
                u0[0:65, :], st["vS0"][:, kc, :], pt[:, 0:512],
                start=(kc == 0), stop=(kc == 15),
            )
            nc.tensor.matmul(
                u1[0:65, :], st["vS1"][:, kc, :], pt[:, 512:1024],
                start=(kc == 0), stop=(kc == 15),
            )

        def pair(b, qt, kc):
            """One score-pair + its exp; returns the pt tile."""
            st = state[b]
            qT, kT = st["q"], st["k"]
            q0, q1 = qt * 512, (qt + 1) * 512
            k0 = kc * 128
            sp = psS.tile([128, 1024], F32, tag="psS", name=f"sp_{b}_{qt}_{kc}")
            nc.tensor.matmul(
                sp[:, 0:512], kT[0:64, k0 : k0 + 128], qT[0:64, q0:q1],
                start=True, stop=True, tile_position=(0, 0),
            )
            nc.tensor.matmul(
                sp[:, 512:1024], kT[64:128, k0 : k0 + 128], qT[64:128, q0:q1],
                start=True, stop=True, tile_position=(64, 0),
            )
            pt = ppt.tile([128, 1024], BF16, tag="pt", name=f"pt_{b}_{qt}_{kc}")
            nc.scalar.activation(pt, sp, EXP, scale=float(SCALE))
            return pt

        def b_block(b, qt, it, nb, early, late, pre, nxt):
            """scores^T -> exp -> attention for one q-tile, with filler
            pieces interleaved into the ACT-paced beats.  The AV matmuls run
            one beat behind the score pairs so nothing in the PE stream
            waits on the just-issued pair.  `pre` is this iteration's beat-0
            pair/exp (already emitted at the tail of the previous
            iteration); `nxt` = (b', qt') requests the same favor forward —
            emitted before the last AV so the scalar engine never idles at
            the iteration boundary.  `early` fillers spread over beats 1-13,
            `late` (norm muls + out-projection, which need ~3 beats of DMA
            lead time) over beats 8-15."""
            st = state[b]
            u0 = psU.tile([128, 512], F32, tag="u", name=f"u0_{it}")
            u1 = psU.tile([128, 512], F32, tag="u", name=f"u1_{it}")
            ei = li = 0
            if pre is not None:
                pts = {0: pre[0], 1: pre[1]}
            else:
                pts = {0: pair(b, qt, 0), 1: pair(b, qt, 1)}
            handoff = None
            for kc in range(2, 18):
                if kc < 16:
                    pts[kc] = pair(b, qt, kc)
                elif nxt is not None:
                    # emit the next iteration's beat-0/1 pair+exp now so the
                    # scalar engine rolls across the boundary without a gap
                    handoff = pair(nxt[0], nxt[1], kc - 16) if kc == 16 else [
                        handoff, pair(nxt[0], nxt[1], 1)]
                av(st, u0, u1, pts.pop(kc - 2), kc - 2)
                if kc == 2 and nb is not None:
                    nb()
                n_e = (len(early) * min(kc - 1, 13)) // 13 - ei
                for _ in range(n_e):
                    early[ei]()
                    ei += 1
                if kc >= 10:
                    n_l = (len(late) * min(kc - 9, 6)) // 6 - li
                    for _ in range(n_l):
                        late[li]()
                        li += 1
            nst = {}
            # copy out of PSUM right away so the u slots free for the next
            # q-tile; the norm chain continues on SBUF tiles.
            for h, u in ((0, u0), (1, u1)):
                usb = pusb.tile([65, 512], F32, tag="usb", name=f"usb{h}_{it}")
                nc.vector.tensor_copy(usb, u[0:65, :])
                nst[f"usb{h}"] = usb
            d2 = dsc.tile([2, 512], F32, tag="d2", name=f"d2_{it}")
            nc.sync.dma_start(d2[0:1, :], nst["usb0"][64:65, :])
            nc.sync.dma_start(d2[1:2, :], nst["usb1"][64:65, :])
            dg = pdg.tile([128, 8], F32, tag="dg", name=f"dg_{it}")
            nc.sync.dma_start(dg, d2[:, :].rearrange("a (x c) -> (a x) c", x=64))
            nst["dg"] = dg
            norm[it] = nst
            return handoff

        # prologue: only batch 0 / q-tile 0's projections before the first
        # b_block.  b_block(0,0) consumes ALL of batch 0's K/V chunks, so
        # the remaining batch-0 quarters are fillers inside t=0 itself (the
        # piece order — k,v,transposes,q per quarter — lands each chunk
        # before the beat that reads it); batch 1's quarters run at t=2..3
        # (its x lands during t=1).  x-DMA emission is staged so the fair-
        # sharing queue delivers each wave just before its readers.
        for p in a_pieces(0, 0):
            p()
        a_allocx(0, half=1)
        extra = {
            0: a_pieces(0, 1) + a_pieces(0, 2) + a_pieces(0, 3),
            2: a_pieces(1, 0) + a_pieces(1, 1),
            3: a_pieces(1, 2) + a_pieces(1, 3),
        }
        handoff = None
        for t in range(B * 4):
            b, qt = divmod(t, 4)
            nb = norm_back(t - 1)[0] if t >= 1 else None
            early = list(extra.get(t, []))
            if b + 1 < B and b >= 1:
                early += a_pieces(b + 1, qt)
            late = c_pieces((t - 1) // 4, (t - 1) % 4, t - 1) if t >= 1 else []
            nxt = divmod(t + 1, 4) if t + 1 < B * 4 else None
            handoff = b_block(b, qt, t, nb, early, late, handoff, nxt)
            if t == 0:
                a_allocx(1, half=0)
                a_allocx(1, half=1)
                a_alloct(1)
            elif t == 1:
                a_allocx(2)
            elif t == 3:
                a_alloct(2)
            elif t == 5:
                a_allocx(3)
            elif t == 7:
                a_alloct(3)
        for p in norm_back(B * 4 - 1):
            p()
        for p in c_pieces(B - 1, 3, B * 4 - 1):
            p()

    return nc


_NC_CACHE = {}


def _get_nc(with_bias=True, probe=None, out_bf16=True):
    key = (with_bias, probe, out_bf16)
    if key not in _NC_CACHE:
        _NC_CACHE[key] = build_attention_nc(with_bias, probe, out_bf16)
    return _NC_CACHE[key]


def _run(inputs, Wq, bq, Wk, bk, Wv, bv, Wo, bo, trace=False, **spmd_kwargs):
    X2 = np.asarray(inputs, dtype=np.float32).reshape(T, D)
    xT = X2.T.astype(NP_BF16).reshape(KC, 128, T)
    with_bias = bool(
        np.any(np.asarray(bq)) or np.any(np.asarray(bk)) or np.any(np.asarray(bv))
    )

    def wprep(W, cs):
        # [D, 128] -> [KC, 128, 128] -> partition-major [128, KC, 128]
        return np.ascontiguousarray(
            np.asarray(W[:, cs]).reshape(KC, 128, 128).transpose(1, 0, 2)
        ).astype(NP_BF16)

    in_maps = []
    for c in range(N_CORES):
        cs = slice(c * 128, (c + 1) * 128)
        in_maps.append(
            {
                "xT": xT,
                "wq": wprep(Wq, cs),
                "wk": wprep(Wk, cs),
                "wv": wprep(Wv, cs),
                "bq": np.asarray(bq[cs]).astype(NP_BF16),
                "bk": np.asarray(bk[cs]).astype(NP_BF16),
                "bv": np.asarray(bv[cs]).astype(NP_BF16),
                "wo": np.ascontiguousarray(Wo[cs, :]).astype(NP_BF16),
            }
        )

    if not with_bias:
        for m in in_maps:
            m.pop("bq"), m.pop("bk"), m.pop("bv")
    res = run_bass_kernel_spmd(
        _get_nc(with_bias), in_maps, list(range(N_CORES)), trace=trace, **spmd_kwargs
    )
    acc = res.results[0]["out"].astype(np.float32)
    for c in range(1, N_CORES):
        acc += res.results[c]["out"]
    acc += np.asarray(bo, dtype=np.float32)[None, :]
    return acc.reshape(B, S, D), res


def kernel(inputs, Wq, bq, Wk, bk, Wv, bv, Wo, bo):
    out, _ = _run(inputs, Wq, bq, Wk, bk, Wv, bv, Wo, bo)
    return out

